# revision 30
# baseline (speedup 1.0000x reference)
"""Trainium2 Bass kernel for nn_MultiHeadLatentAttention_82068235092052.

Reference computation (B=2, S=4096, E=4096, H=32, D=128):
    q = hs @ wq.T + bq   -> [B,S,H,D]     (wq/bq are fp8-roundtripped fp32)
    k = hs @ wk.T + bk
    v = hs @ wv.T + bv
    (latent = hs @ wl.T + bl is computed but UNUSED -> skipped entirely)
    scores  = einsum('bshd,bstd->bsht', q, k) / sqrt(D)   # attention over HEADS per position
    probs   = softmax(scores, -1)
    context = einsum('bsht,bstd->bshd', probs, v).reshape(B,S,E)

Strategy: data-parallel over the 8192 positions across 8 cores (1024 each).
Per core the positions are processed in 5 slabs (256/256/256/192/64); the
per-position 32x32 head-attention of slab s-1 is interleaved into the
projection matmuls of slab s, so only the tiny last slab's attention is
exposed at the end.

Projections stream the fused W[12288,4096] weights as fp8-e4m3 (lossless:
the reference weights are fp8-roundtripped) into mixed fp8xbf16 matmuls.
q/k/v land pos-major [d, pos, head]; per 16-position block the scores are
computed with 4 cross-position matmuls (N=128, 4 positions each), the
off-diagonal cross terms are killed by a block-diagonal mask fused into the
tensor_tensor_reduce that also produces the softmax denominators (zsum).
probs stay UNNORMALIZED on device; zsum ships to the host, which divides in
fp32 while assembling the output.
"""

import os
import sys

import numpy as np

sys.path.insert(0, "/opt/trn_rl_repo")

import ml_dtypes

import concourse.bacc as bacc
import concourse.bass as bass
import concourse.tile as tile
from concourse import mybir
from concourse.masks import make_identity

# Problem constants (hardcoded; kernel.py must be self-contained).
B, S, E = 2, 4096, 4096
H, D = 32, 128
P_TOT = B * S            # 8192 positions
N_CORES = 8
P_CORE = P_TOT // N_CORES  # 1024 positions per core
FT = 3 * H                 # 96 feature tiles (q, k, v concatenated)
KT = E // 128              # 32 contraction tiles

SLABS = [288, 288, 288, 160]
assert sum(SLABS) == P_CORE and all(s % 16 == 0 for s in SLABS)
SMAX = max(SLABS)

BF16 = mybir.dt.bfloat16
F32 = mybir.dt.float32
FP8 = mybir.dt.float8e4

_CACHED_NC = None


def build_nc():
    """Build the per-core Bass program (same program on all 8 cores)."""
    nc = bacc.Bacc(
        "TRN2",
        target_bir_lowering=False,
        debug=False,
        enable_asserts=True,
        num_devices=1,
    )

    xt = nc.dram_tensor("xt", [128, KT, P_CORE], BF16, kind="ExternalInput").ap()
    wt = nc.dram_tensor("wt", [FT, 128, KT * 128], FP8, kind="ExternalInput").ap()
    bias = nc.dram_tensor("bias", [128, FT], F32, kind="ExternalInput").ap()
    maskd = nc.dram_tensor("maskd", [128, 128], BF16, kind="ExternalInput").ap()
    ctx_out = nc.dram_tensor("ctx", [128, P_CORE, H], BF16, kind="ExternalOutput").ap()
    zsum_out = nc.dram_tensor("zsum", [128, P_CORE // 4], F32, kind="ExternalOutput").ap()

    from contextlib import ExitStack

    with tile.TileContext(nc) as tc, ExitStack() as stack:
        const = stack.enter_context(tc.tile_pool(name="const", bufs=1))
        xtp = stack.enter_context(tc.tile_pool(name="xtp", bufs=2))
        qkvp = stack.enter_context(tc.tile_pool(name="qkvp", bufs=2))
        wp = stack.enter_context(tc.tile_pool(name="wp", bufs=3))
        zsp = stack.enter_context(tc.tile_pool(name="zsp", bufs=2))
        asb = stack.enter_context(tc.tile_pool(name="asb", bufs=2))
        ctp = stack.enter_context(tc.tile_pool(name="ctp", bufs=3))
        psp = stack.enter_context(tc.tile_pool(name="psp", bufs=2, space="PSUM"))
        scp = stack.enter_context(tc.tile_pool(name="scp", bufs=2, space="PSUM"))
        vtp = stack.enter_context(tc.tile_pool(name="vtp", bufs=2, space="PSUM"))
        cdp = stack.enter_context(tc.tile_pool(name="cdp", bufs=2, space="PSUM"))

        identity = const.tile([128, 128], BF16)
        make_identity(nc, identity)
        bias_sb = const.tile([128, FT], F32)
        nc.sync.dma_start(bias_sb, bias)
        mask_sb = const.tile([128, 128], BF16)
        nc.sync.dma_start(mask_sb, maskd)

        inv_sqrt_d = 1.0 / float(np.sqrt(D))

        def emit_attn_front(slab_tiles, blk):
            """QK + VT + softmax front half of one 16-position block."""
            q_sb, k_sb, v_sb, zs_sb, sstart = slab_tiles
            p0 = blk * 16
            scores = scp.tile([128, 4, 128], F32, tag="sc")
            for g in range(4):
                nc.tensor.matmul(
                    scores[:, g, :],
                    lhsT=q_sb[:, p0 + 4 * g:p0 + 4 * g + 4, :],
                    rhs=k_sb[:, p0 + 4 * g:p0 + 4 * g + 4, :],
                    start=True,
                    stop=True,
                )
            vt_ps = vtp.tile([128, 4, 128], BF16, tag="vt", padded_shape=[128, 4, 256])
            for g in range(4):
                nc.tensor.transpose(
                    vt_ps[:, g, :],
                    v_sb[:, p0 + 4 * g:p0 + 4 * g + 4, :].opt(),
                    identity,
                )
            exp_sb = asb.tile([128, 4, 128], BF16, tag="exp")
            nc.scalar.activation(
                exp_sb, scores, mybir.ActivationFunctionType.Exp, scale=inv_sqrt_d
            )
            masked = asb.tile([128, 4, 128], BF16, tag="mk")
            nc.vector.tensor_tensor(
                masked,
                exp_sb,
                mask_sb[:, None, :].to_broadcast((128, 4, 128)),
                mybir.AluOpType.mult,
            )
            nc.vector.tensor_reduce(
                zs_sb[:, 4 * blk:4 * blk + 4],
                masked,
                axis=mybir.AxisListType.X,
                op=mybir.AluOpType.add,
            )
            probsT = asb.tile([128, 4, 128], BF16, tag="pt")
            nc.vector.transpose(probsT, masked)  # block-diagonal -> true transpose
            vt_sb = asb.tile([128, 4, 128], BF16, tag="vts")
            nc.scalar.copy(vt_sb, vt_ps)
            return (probsT, vt_sb, sstart, p0)

        def emit_attn_back(pend):
            """PV + ctx output of a previously fronted block."""
            probsT, vt_sb, sstart, p0 = pend
            ctd = cdp.tile([128, 4, 128], F32, tag="ctd")
            for g in range(4):
                nc.tensor.matmul(
                    ctd[:, g, :],
                    lhsT=vt_sb[:, g, :],
                    rhs=probsT[:, g, :],
                    start=True,
                    stop=True,
                )
            ct_blk = ctp.tile([128, 16, H], BF16, tag="ct")
            nc.vector.tensor_scalar(
                out=ct_blk.rearrange("d a b -> d (a b)"),
                in0=ctd.rearrange("d a b -> d (a b)"),
                scalar1=0.0,
                scalar2=None,
                op0=mybir.AluOpType.add,
            )
            nc.sync.dma_start(
                ctx_out[:, sstart + p0:sstart + p0 + 16, :], ct_blk
            )

        def fetch_xt(si):
            s0 = sum(SLABS[:si])
            xt_sb = xtp.tile([128, KT, SLABS[si]], BF16, tag="xt")
            nch = 8 if si == 0 else 4
            w = KT // nch
            for kc in range(nch):
                nc.sync.dma_start(
                    xt_sb[:, w * kc:w * kc + w, :],
                    xt[:, w * kc:w * kc + w, s0:s0 + SLABS[si]],
                )
            return xt_sb

        warm = psp.tile([128, 128], F32, tag="ps", padded_shape=[128, 512])
        for _ in range(24):
            nc.tensor.matmul(warm, lhsT=identity, rhs=identity, start=True, stop=True)

        prev_tiles = None
        pending = None
        sstart = 0
        next_xt = fetch_xt(0)
        for si, SL in enumerate(SLABS):
            xt_sb = next_xt
            q_sb = qkvp.tile([128, SL, H], BF16, tag="q")
            k_sb = qkvp.tile([128, SL, H], BF16, tag="k")
            v_sb = qkvp.tile([128, SL, H], BF16, tag="v")
            zs_sb = zsp.tile([128, SL // 4], F32, tag="zs")
            dsts = (q_sb, k_sb, v_sb)

            nblk_prev = SLABS[si - 1] // 16 if si > 0 else 0
            attn_j = 0
            for ft in range(FT):
                w_sb = wp.tile([128, KT, 128], FP8, tag="w")
                wsrc = wt[ft].rearrange("p (a b) -> p a b", a=KT)
                if si == 0 and ft == 0:
                    for kc in range(4):
                        nc.sync.dma_start(
                            w_sb[:, 8 * kc:8 * kc + 8, :], wsrc[:, 8 * kc:8 * kc + 8, :]
                        )
                else:
                    nc.sync.dma_start(w_sb, wsrc)
                ps = psp.tile([128, SL], F32, tag="ps", padded_shape=[128, 512])
                for kt in range(KT):
                    nc.tensor.matmul(
                        ps,
                        lhsT=w_sb[:, kt, :],
                        rhs=xt_sb[:, kt, :],
                        start=(kt == 0),
                        stop=(kt == KT - 1),
                    )
                # bias add (per-partition scalar) + cast to bf16, PSUM -> SBUF
                nc.vector.tensor_scalar(
                    out=dsts[ft // H][:, :, ft % H],
                    in0=ps,
                    scalar1=bias_sb[:, ft:ft + 1],
                    scalar2=None,
                    op0=mybir.AluOpType.add,
                )
                if ft == 8 and si + 1 < len(SLABS):
                    next_xt = fetch_xt(si + 1)
                # interleave previous slab's attention blocks across the ft loop
                while attn_j < min(nblk_prev, nblk_prev * (ft + 1) * 3 // (2 * FT)):
                    front = emit_attn_front(prev_tiles, attn_j)
                    if pending is not None:
                        emit_attn_back(pending)
                    pending = front
                    attn_j += 1
            if si > 0:
                nc.sync.dma_start(
                    zsum_out[:, prev_tiles[4] // 4:(prev_tiles[4] + SLABS[si - 1]) // 4],
                    prev_tiles[3],
                )
            prev_tiles = (q_sb, k_sb, v_sb, zs_sb, sstart)
            sstart += SL

        # last slab's attention (the only non-overlapped part)
        for blk in range(SLABS[-1] // 16):
            front = emit_attn_front(prev_tiles, blk)
            if pending is not None:
                emit_attn_back(pending)
            pending = front
        emit_attn_back(pending)
        nc.sync.dma_start(
            zsum_out[:, prev_tiles[4] // 4:(prev_tiles[4] + SLABS[-1]) // 4],
            prev_tiles[3],
        )

    nc.compile()
    return nc


def get_nc():
    global _CACHED_NC
    if _CACHED_NC is None:
        _CACHED_NC = build_nc()
    return _CACHED_NC


def prep_inputs(hidden_states, wq, bq, wk, bk, wv, bv):
    """Host-side layout prep. Returns per-core input maps."""
    bf16 = ml_dtypes.bfloat16

    # X^T tiled: [ipart, kt, p] with p the global position index
    xt_all = (
        np.ascontiguousarray(hidden_states.reshape(P_TOT, E).T)
        .astype(bf16)
        .reshape(KT, 128, P_TOT)
        .transpose(1, 0, 2)
    )  # [128, KT, 8192] (view)

    # Fused weight W[12288, 4096] -> W^T tiled [ft, ipart, kt*128 + f].
    # Weights are fp8-e4m3 roundtripped, so fp8 storage is lossless.
    wcat = np.concatenate([wq, wk, wv], axis=0)  # [3E, E]
    wt = (
        np.ascontiguousarray(wcat.T)
        .astype(ml_dtypes.float8_e4m3)
        .reshape(KT, 128, FT, 128)
        .transpose(2, 1, 0, 3)
    )
    wt = np.ascontiguousarray(wt).reshape(FT, 128, KT * 128)

    bias_cols = np.ascontiguousarray(
        np.concatenate([bq, bk, bv]).astype(np.float32).reshape(FT, 128).T
    )  # [128, FT]

    mask = np.zeros((128, 128), dtype=bf16)
    for p in range(4):
        mask[32 * p:32 * p + 32, 32 * p:32 * p + 32] = 1.0

    in_maps = []
    for c in range(N_CORES):
        xt_c = np.ascontiguousarray(xt_all[:, :, c * P_CORE:(c + 1) * P_CORE])
        in_maps.append({"xt": xt_c, "wt": wt, "bias": bias_cols, "maskd": mask})
    return in_maps


def normalize_shard(ctx_u, zsum):
    """ctx_u [128, P_CORE, H] bf16 (d, pos, h) unnormalized; zsum [128, P_CORE//4].

    Returns normalized [P_CORE, E] fp32. z for position pos, head h lives at
    zsum[32*(pos%4)+h, pos//4]."""
    ctx = np.asarray(ctx_u).astype(np.float32).transpose(1, 2, 0)  # [pos, h, d]
    z = np.asarray(zsum).astype(np.float32)  # [128, P_CORE//4]
    z = z.reshape(4, 32, P_CORE // 4).transpose(2, 0, 1).reshape(P_CORE, 32)
    return (ctx / z[:, :, None]).reshape(P_CORE, E)


def assemble_output(ctxs, zsums):
    out = np.empty((P_TOT, E), dtype=np.float32)
    for c in range(N_CORES):
        out[c * P_CORE:(c + 1) * P_CORE] = normalize_shard(ctxs[c], zsums[c])
    return out.reshape(B, S, E)


def kernel(**inputs):
    from concourse.bass_utils import run_bass_kernel_spmd

    nc = get_nc()
    in_maps = prep_inputs(
        inputs["hidden_states"],
        inputs["wq"], inputs["bq"],
        inputs["wk"], inputs["bk"],
        inputs["wv"], inputs["bv"],
    )
    res = run_bass_kernel_spmd(nc, in_maps, core_ids=list(range(N_CORES)))
    ctxs = [np.asarray(r["ctx"]).reshape(128, P_CORE, H) for r in res.results]
    zsums = [np.asarray(r["zsum"]).reshape(128, P_CORE // 4) for r in res.results]
    return assemble_output(ctxs, zsums)
